# revision 36
# baseline (speedup 1.0000x reference)
"""Trainium2 Bass kernel: decode-step attention with static KV cache (GQA).

Problem shapes (hardcoded):
  x        [16, 1, 4096]      activations (B=16, QLEN=1, DIM=4096)
  cache_k  [16, 8192, 8, 128] K cache (PREFIX=8192, HKV=8, HD=128)
  cache_v  [16, 8192, 8, 128]
  wq       [4096, 4096]  (H*HD, DIM), H=32
  wk/wv    [1024, 4096]
  wo       [4096, 4096]  (DIM, H*HD)
  out      [16, 1, 4096]

Sharding: tensor-parallel over the kv-head axis. Core c owns kv head c and
q heads 4c..4c+3; weights are column/row-sliced per core, the KV slice is
extracted per core on the host. Each core computes a partial [16, 4096]
output (stored transposed as outT [128, 32*16]); the host sums the 8
partials.

The kernel is HBM-bandwidth bound, so every input is cast to float16 ON
THE HOST and pre-swizzled into the exact SBUF layout the kernel wants:
all device DMAs are then plain contiguous HWDGE loads (nc.sync), K+V for
one batch are fused into a single 4 MiB transfer, and HBM bytes per core
drop to ~74 MiB (K 32 + V 32 + weights ~10).  f32 numerics only in PSUM
accumulation; identical rounding to the old in-DMA-cast scheme.

t-ordering: V loads contiguously as [128, (n d)] with t = 64*p + n
(p = partition, n = tile index).  The host permutes K's columns to the
same order, so score tiles and V tiles agree on partition<->t mapping.

Matmul orientation is chosen so the stationary operand always has 128
columns (FWL) and the moving operand is small:
  q/k_new proj:  w-tile stationary, x [128,16] moving -> qT/kT directly.
  scores:        K-tile stationary, q [128,4] moving -> scoresT [t',h].
  PV:            V-tile stationary, P [128,4] moving -> out [d,h] (no
                 transpose needed; softmax scale applied per-column via a
                 ones-outer-product broadcast matmul + elementwise mult).
  out proj:      wo-tile stationary, AT [128,16] moving -> outT [n,b].

DMA stream order is made structural (the Tile scheduler reorders
dependency-free DMAs, e.g. pushing a standalone wo load to the very end
of the stream where it starves the tail):
  - wo rides embedded inside the other loads: 1024 columns appended to
    each fused KV load (b 0..13) + 2048 to the wq load; tiny DVE copies
    unpack them into the resident wo tile.
  - batches 14..15 issue K and V separately on the second HWDGE ring
    (ACT), where they serialize in need-order behind the exp chain and
    bypass the SP ring's backlog; scores(15) gates on K15 alone.
  - denominator matmuls run BEFORE the PV loop (they only need P), so
    the reciprocal broadcast is ready when the last V lands.

Per-core dataflow:
  phase 0: load xs/wk/wv/wq (f16, contiguous), project q/k_new/v_new.
  phase 1 (per b): one fused 4.4MiB K+V+wo load; 64+1 score matmuls ->
           PSUM f32; exp (ACT, scale=1/sqrt(128)) -> P f16; denominators
           + reciprocal broadcast; 64+1 PV matmuls accumulate [d, h] in
           PSUM f32; scaled write into AT[d,(h,b)] f16.
  phase 2: outT[n-tile, b] = wo-tiles.T @ AT-slices, 4 chunked stores.
"""

import sys

_REPO = "/opt/trn_rl_repo"
if _REPO not in sys.path:
    sys.path.insert(0, _REPO)

import numpy as np

import concourse.bacc as bacc
import concourse.mybir as mybir
import concourse.tile as tile
from concourse.bass_utils import run_bass_kernel_spmd

B = 16          # batch
T = 8192        # prefix length in cache
NT = T // 128   # 64 K/V tiles per batch
HD = 128        # head dim
HQ = 4          # q heads per core
DIM = 4096
NDT = DIM // 128  # 32 contraction tiles for the projections
NCORES = 8
F32 = mybir.dt.float32
F16 = mybir.dt.float16
SCALE = 1.0 / float(np.sqrt(128.0))
SW = 4 * NT + 4   # score tile width: 64 cache tiles + new token, 4 heads each
WC = 1024         # wo columns carried by each fused KV load (b 0..13)

Exp = mybir.ActivationFunctionType.Exp


def _build_nc():
    nc = bacc.Bacc("TRN2", target_bir_lowering=False, debug=False)

    # wo rides inside the wq / fused-KV loads (WC-col chunks appended to
    # each): the scheduler otherwise reorders a standalone wo DMA to the
    # very end of the stream, where it starves the tail.
    xs = nc.dram_tensor("xs", [128, NDT * B], F16, kind="ExternalInput")
    wqs = nc.dram_tensor(
        "wqs", [128, NDT * HQ * HD + 2 * WC], F16, kind="ExternalInput"
    )
    wks = nc.dram_tensor("wks", [128, NDT * HD], F16, kind="ExternalInput")
    wvs = nc.dram_tensor("wvs", [128, NDT * HD], F16, kind="ExternalInput")
    kv = nc.dram_tensor("kv", [B, 128, 2 * T + WC], F16, kind="ExternalInput")
    outT = nc.dram_tensor("outT", [128, NDT * B], F32, kind="ExternalOutput")

    with tile.TileContext(nc) as tc:
        _emit(nc, tc, xs, wqs, wks, wvs, kv, outT)
    nc.compile()
    return nc


def _emit(nc, tc, xs, wqs, wks, wvs, kv, outT):
    from contextlib import ExitStack

    with ExitStack() as ctx:
        const = ctx.enter_context(tc.tile_pool(name="const", bufs=1))

        # x^T in f16: [128, (dt b)], host pre-swizzled, contiguous load
        xs_h = const.tile([128, NDT * B], F16, tag="xs_h")
        nc.sync.dma_start(xs_h[:], xs[:])

        # wk/wv resident f16 [128, (dt n)]
        wk_h = const.tile([128, NDT * HD], F16, tag="wk_h")
        nc.sync.dma_start(wk_h[:], wks[:])
        wv_h = const.tile([128, NDT * HD], F16, tag="wv_h")
        nc.sync.dma_start(wv_h[:], wvs[:])

        QTh = const.tile([128, HQ * B], F16, tag="QTh")     # q^T [d,(h,b)] f16
        KTnh = const.tile([128, B], F16, tag="KTnh")        # new-token K^T f16
        vrowh = const.tile([1, B * HD], F16, tag="vrowh")   # new-token V rows f16
        AT = const.tile([128, HQ * B], F16, tag="AT")       # attn out^T f16
        wo_h = const.tile([128, HQ * DIM], F16, tag="wo_h") # resident f16 wo
        vn_h = const.tile([B, HD], F16, tag="vn_h")
        outs = const.tile([128, NDT * B], F32, tag="outs")  # outT staging
        ones_h = const.tile([128, 1], F16, tag="ones_h")    # denominator lhsT
        ones_r = const.tile([1, 128], F32, tag="ones_r")    # bcast-matmul lhsT

        nc.vector.memset(ones_h[:], 1.0)
        nc.vector.memset(ones_r[:], 1.0)

        # ---------------- phase 0: projections (f16 PE, weights stationary) --
        with tc.tile_pool(name="psum0", bufs=1, space="PSUM") as pp0:
            qtp = [
                pp0.tile([128, B], F32, tag=f"qtp{h}", name=f"qtp{h}")
                for h in range(HQ)
            ]
            ktp = pp0.tile([128, B], F32, tag="ktp")
            vnp = pp0.tile([B, HD], F32, tag="vnp")

            # one wq load (+2 embedded wo chunks): 36KB/partition contiguous.
            # Resident in const: a closable pool would let kvpool reuse the
            # address space, which serializes KV0's write behind the
            # q-projection reads (~8us stall at the head of the stream).
            NWQ = NDT * HQ * HD
            wq_h = const.tile([128, NWQ + 2 * WC], F16, tag="wq_h")
            nc.sync.dma_start(wq_h[:], wqs[:])
            nc.vector.tensor_copy(
                wo_h[:, 14 * WC:16 * WC], wq_h[:, NWQ:NWQ + 2 * WC]
            )
            for h in range(HQ):
                for dt in range(NDT):
                    nc.tensor.matmul(
                        qtp[h][:],
                        wq_h[:, dt * HQ * HD + h * HD:
                             dt * HQ * HD + (h + 1) * HD],
                        xs_h[:, dt * B:(dt + 1) * B],
                        start=(dt == 0), stop=(dt == NDT - 1),
                    )
            for dt in range(NDT):
                nc.tensor.matmul(
                    ktp[:], wk_h[:, dt * HD:(dt + 1) * HD],
                    xs_h[:, dt * B:(dt + 1) * B],
                    start=(dt == 0), stop=(dt == NDT - 1),
                )
            for dt in range(NDT):
                nc.tensor.matmul(
                    vnp[:], xs_h[:, dt * B:(dt + 1) * B],
                    wv_h[:, dt * HD:(dt + 1) * HD],
                    start=(dt == 0), stop=(dt == NDT - 1),
                )

            for h in range(HQ):
                nc.vector.tensor_copy(QTh[:, h * B:(h + 1) * B], qtp[h][:])
            nc.vector.tensor_copy(KTnh[:], ktp[:])
            nc.vector.tensor_copy(vn_h[:], vnp[:])

            # v_new rows flattened onto partition 0 (HWDGE SBUF->SBUF; no
            # SWDGE anywhere -> SDMA engine 15 keeps its full bandwidth).
            # On the ACT ring: its data-dependency wait (v-projection chain)
            # must not stall the SP ring ahead of the first KV load.
            nc.scalar.dma_start(
                vrowh[:].rearrange("p (b c) -> p b c", c=HD)[0:1, :, :],
                vn_h[:],
            )

        # ---------------- phase 1: attention over the cache ----------------
        QTh3 = QTh[:].rearrange("p (h b) -> p b h", b=B)   # [128, b, 4]
        vrowh3 = vrowh[:].rearrange("p (b c) -> p b c", c=HD)
        AT3 = AT[:].rearrange("p (h b) -> p b h", b=B)

        with (
            tc.tile_pool(name="kvpool", bufs=3) as kvpool,
            tc.tile_pool(name="ptpool", bufs=2) as ptpool,
            tc.tile_pool(name="small", bufs=2) as small,
            tc.tile_pool(name="stpsum", bufs=2, space="PSUM") as stpsum,
            tc.tile_pool(name="opsum", bufs=2, space="PSUM") as opsum,
            tc.tile_pool(name="denpsum", bufs=1, space="PSUM") as denpsum,
            tc.tile_pool(name="rcbpsum", bufs=1, space="PSUM") as rcbpsum,
        ):
            # Batches 0..13: one fused 4MiB KV load each on the SP ring
            # (32KB/partition -> 32KB packets, best per-engine rate).
            # Batches 14..15: K and V split, issued on the second HWDGE
            # ring (ACT).  Their triggers sit behind exp(13)/exp(14) in ACT
            # program order, which serializes the tail loads in exactly
            # need-order right behind the SP stream's end.
            for b in range(B):
                kvb = kvpool.tile([128, 2 * T + WC], F16, tag="kvb",
                                  name=f"kvb{b}")
                if b < 14:
                    nc.sync.dma_start(kvb[:], kv[b])
                    # unpack this load's embedded wo chunk (rides the fused
                    # DMA so the scheduler cannot reorder it to the end)
                    nc.vector.tensor_copy(
                        wo_h[:, b * WC:(b + 1) * WC],
                        kvb[:, 2 * T:2 * T + WC],
                    )
                else:
                    nc.scalar.dma_start(kvb[:, 0:T], kv[b][:, 0:T])
                    nc.scalar.dma_start(kvb[:, T:2 * T], kv[b][:, T:2 * T])
                ktb = kvb[:, 0:T]
                vb = kvb[:, T:2 * T]

                # scores^T tiles: [t'(128), h(4)] per cache tile + new token
                stp = stpsum.tile([128, SW], F32, tag="stp")
                qb = QTh3[:, b, :]
                nc.tensor.matmul(
                    stp[0:1, 4 * NT:SW], KTnh[:, b:b + 1], qb,
                    start=True, stop=True,
                )
                for n in range(NT):
                    nc.tensor.matmul(
                        stp[:, 4 * n:4 * n + 4],
                        ktb[:, 128 * n:128 * (n + 1)],
                        qb,
                        start=True, stop=True,
                    )

                pt = ptpool.tile([128, SW], F16, tag="pt")
                nc.scalar.activation(pt[:, 0:4 * NT], stp[:, 0:4 * NT], Exp,
                                     scale=SCALE)
                nc.scalar.activation(
                    pt[0:1, 4 * NT:SW], stp[0:1, 4 * NT:SW], Exp, scale=SCALE,
                )

                # softmax denominators first (need only pt, not V): by the
                # time V lands the reciprocal broadcast is already done.
                dps = denpsum.tile([1, SW], F32, tag="dps")
                nc.tensor.matmul(
                    dps[0:1, 0:4 * NT], ones_h[:], pt[:, 0:4 * NT],
                    start=True, stop=True,
                )
                nc.tensor.matmul(
                    dps[0:1, 4 * NT:SW], ones_h[0:1, 0:1], pt[0:1, 4 * NT:SW],
                    start=True, stop=True,
                )
                dred = small.tile([1, HQ], F32, tag="dred")
                nc.vector.reduce_sum(
                    dred[:].rearrange("p h -> p h ()"),
                    dps[:].rearrange("p (g h) -> p h g", h=HQ),
                    axis=mybir.AxisListType.X,
                )
                rcr = small.tile([1, HQ], F32, tag="rcr")
                nc.vector.reciprocal(rcr[:], dred[:])
                # broadcast 1/den across partitions: ones[128] outer rcr[4]
                rcb = rcbpsum.tile([128, HQ], F32, tag="rcb")
                nc.tensor.matmul(rcb[:], ones_r[:], rcr[:],
                                 start=True, stop=True)
                rcs = small.tile([128, HQ], F32, tag="rcs")
                nc.vector.tensor_copy(rcs[:], rcb[:])

                # out [d(128), h(4)]: V tiles stationary (FWL), P moving
                op = opsum.tile([128, HQ], F32, tag="op")
                nc.tensor.matmul(
                    op[:], vrowh3[0:1, b, :], pt[0:1, 4 * NT:SW],
                    start=True, stop=False,
                )
                for n in range(NT):
                    nc.tensor.matmul(
                        op[:],
                        vb[:, 128 * n:128 * (n + 1)],
                        pt[:, 4 * n:4 * n + 4],
                        start=False, stop=(n == NT - 1),
                    )
                nc.vector.tensor_mul(AT3[:, b, :], op[:], rcs[:])

        # ---------------- phase 2: output projection (wo stationary) -------
        with tc.tile_pool(name="wopsum", bufs=4, space="PSUM") as wps:
            for nt in range(NDT):
                wop = wps.tile([128, B], F32, tag="wop")
                for cc in range(HQ):
                    nc.tensor.matmul(
                        wop[:],
                        wo_h[:, cc * DIM + nt * 128:cc * DIM + (nt + 1) * 128],
                        AT[:, cc * B:(cc + 1) * B],
                        start=(cc == 0), stop=(cc == HQ - 1),
                    )
                nc.vector.tensor_copy(outs[:, nt * B:(nt + 1) * B], wop[:])
                # stream the store out in 4 chunks as tiles complete
                if nt % 8 == 7:
                    g = nt // 8
                    nc.sync.dma_start(
                        outT[:, g * 8 * B:(g + 1) * 8 * B],
                        outs[:, g * 8 * B:(g + 1) * 8 * B],
                    )


_NC = None


def _get_nc():
    global _NC
    if _NC is None:
        _NC = _build_nc()
    return _NC


def _swz(w):
    """[DIM, n] -> [128, (dt n)] f16, dt-tile-major per partition."""
    n = w.shape[1]
    return np.ascontiguousarray(
        w.reshape(NDT, 128, n).transpose(1, 0, 2).reshape(128, NDT * n)
    ).astype(np.float16)


def make_in_maps(inputs):
    x = np.asarray(inputs["x"], dtype=np.float32).reshape(B, DIM)
    ck = np.asarray(inputs["cache_k"], dtype=np.float32)
    cv = np.asarray(inputs["cache_v"], dtype=np.float32)
    wq = np.asarray(inputs["wq"], dtype=np.float32)
    wk = np.asarray(inputs["wk"], dtype=np.float32)
    wv = np.asarray(inputs["wv"], dtype=np.float32)
    wo = np.asarray(inputs["wo"], dtype=np.float32)

    xs = _swz(np.ascontiguousarray(x.T))
    ck16 = ck.astype(np.float16)
    cv16 = cv.astype(np.float16)

    in_maps = []
    for c in range(NCORES):
        hq0 = HQ * HD * c
        wks = _swz(np.ascontiguousarray(wk[HD * c:HD * (c + 1), :].T))
        wvs = _swz(np.ascontiguousarray(wv[HD * c:HD * (c + 1), :].T))
        wos = np.ascontiguousarray(
            wo[:, hq0:hq0 + HQ * HD].T.reshape(HQ, 128, DIM)
            .transpose(1, 0, 2).reshape(128, HQ * DIM)
        ).astype(np.float16)
        # wq with wo chunks 14..15 appended; KV with wo chunk b appended
        wqs = np.ascontiguousarray(np.concatenate(
            [_swz(np.ascontiguousarray(wq[hq0:hq0 + HQ * HD, :].T)),
             wos[:, 14 * WC:16 * WC]], axis=1
        ))
        # K^T with columns permuted to the t = 64*p + n interleaved order
        # (matches V's natural contiguous-load partition mapping).
        a = ck16[:, :, c, :].reshape(B, 128, NT, HD)       # [b, p, n, d]
        kT_c = a.transpose(0, 3, 2, 1).reshape(B, HD, T)   # [b, d, 128n+p]
        v_c = np.ascontiguousarray(cv16[:, :, c, :]).reshape(B, 128, T)
        wo3 = wos[:, 0:B * WC].reshape(128, B, WC).transpose(1, 0, 2)
        kv_c = np.ascontiguousarray(
            np.concatenate([kT_c, v_c, wo3], axis=2)
        )
        in_maps.append({
            "xs": xs, "wqs": wqs, "wks": wks, "wvs": wvs,
            "kv": kv_c,
        })
    return in_maps


def gather(results):
    """Sum per-core outT partials and undo the [n-tile, b] transpose."""
    acc = np.zeros((B, DIM), dtype=np.float64)
    for r in results:
        o = np.asarray(r["outT"], dtype=np.float64)       # [128, (nt b)]
        acc += o.reshape(128, NDT, B).transpose(2, 1, 0).reshape(B, DIM)
    return acc


def run(in_maps, trace=False):
    nc = _get_nc()
    return run_bass_kernel_spmd(nc, in_maps, list(range(NCORES)), trace=trace)


def kernel(**inputs):
    res = run(make_in_maps(inputs)).results
    return gather(res).astype(np.float32).reshape(B, 1, DIM)


# revision 37
# speedup vs baseline: 1.0102x; 1.0102x over previous
"""Trainium2 Bass kernel: decode-step attention with static KV cache (GQA).

Problem shapes (hardcoded):
  x        [16, 1, 4096]      activations (B=16, QLEN=1, DIM=4096)
  cache_k  [16, 8192, 8, 128] K cache (PREFIX=8192, HKV=8, HD=128)
  cache_v  [16, 8192, 8, 128]
  wq       [4096, 4096]  (H*HD, DIM), H=32
  wk/wv    [1024, 4096]
  wo       [4096, 4096]  (DIM, H*HD)
  out      [16, 1, 4096]

Sharding: tensor-parallel over the kv-head axis. Core c owns kv head c and
q heads 4c..4c+3; weights are column/row-sliced per core, the KV slice is
extracted per core on the host. Each core computes a partial [16, 4096]
output (stored transposed as outT [128, 32*16]); the host sums the 8
partials.

The kernel is HBM-bandwidth bound, so every input is cast to float16 ON
THE HOST and pre-swizzled into the exact SBUF layout the kernel wants:
all device DMAs are then plain contiguous HWDGE loads (nc.sync), K+V for
one batch are fused into a single 4 MiB transfer, and HBM bytes per core
drop to ~74 MiB (K 32 + V 32 + weights ~10).  f32 numerics only in PSUM
accumulation; identical rounding to the old in-DMA-cast scheme.

t-ordering: V loads contiguously as [128, (n d)] with t = 64*p + n
(p = partition, n = tile index).  The host permutes K's columns to the
same order, so score tiles and V tiles agree on partition<->t mapping.

Matmul orientation is chosen so the stationary operand always has 128
columns (FWL) and the moving operand is small:
  q/k_new proj:  w-tile stationary, x [128,16] moving -> qT/kT directly.
  scores:        K-tile stationary, q [128,4] moving -> scoresT [t',h].
  PV:            V-tile stationary, P [128,4] moving -> out [d,h] (no
                 transpose needed; softmax scale applied per-column via a
                 ones-outer-product broadcast matmul + elementwise mult).
  out proj:      wo-tile stationary, AT [128,16] moving -> outT [n,b].

DMA stream order is made structural (the Tile scheduler reorders
dependency-free DMAs, e.g. pushing a standalone wo load to the very end
of the stream where it starves the tail):
  - wo rides embedded inside the other loads: 1024 columns appended to
    each fused KV load (b 0..13) + 2048 to the wq load; tiny DVE copies
    unpack them into the resident wo tile.
  - batches 14..15 issue K and V separately on the second HWDGE ring
    (ACT), where they serialize in need-order behind the exp chain and
    bypass the SP ring's backlog; scores(15) gates on K15 alone.
  - denominator matmuls run BEFORE the PV loop (they only need P), so
    the reciprocal broadcast is ready when the last V lands.

Per-core dataflow:
  phase 0: load xs/wk/wv/wq (f16, contiguous), project q/k_new/v_new.
  phase 1 (per b): one fused 4.4MiB K+V+wo load; 64+1 score matmuls ->
           PSUM f32; exp (ACT, scale=1/sqrt(128)) -> P f16; denominators
           + reciprocal broadcast; 64+1 PV matmuls accumulate [d, h] in
           PSUM f32; scaled write into AT[d,(h,b)] f16.
  phase 2: outT[n-tile, b] = wo-tiles.T @ AT-slices, 4 chunked stores.
"""

import sys

_REPO = "/opt/trn_rl_repo"
if _REPO not in sys.path:
    sys.path.insert(0, _REPO)

import numpy as np

import concourse.bacc as bacc
import concourse.mybir as mybir
import concourse.tile as tile
from concourse.bass_utils import run_bass_kernel_spmd

B = 16          # batch
T = 8192        # prefix length in cache
NT = T // 128   # 64 K/V tiles per batch
HD = 128        # head dim
HQ = 4          # q heads per core
DIM = 4096
NDT = DIM // 128  # 32 contraction tiles for the projections
NCORES = 8
F32 = mybir.dt.float32
F16 = mybir.dt.float16
SCALE = 1.0 / float(np.sqrt(128.0))
SW = 4 * NT + 4   # score tile width: 64 cache tiles + new token, 4 heads each
WC = 1024         # wo columns carried by each fused KV load (b 0..13)

Exp = mybir.ActivationFunctionType.Exp


def _build_nc():
    nc = bacc.Bacc("TRN2", target_bir_lowering=False, debug=False)

    # wo rides inside the wq / fused-KV loads (WC-col chunks appended to
    # each): the scheduler otherwise reorders a standalone wo DMA to the
    # very end of the stream, where it starves the tail.
    xs = nc.dram_tensor("xs", [128, NDT * B], F16, kind="ExternalInput")
    wqs = nc.dram_tensor(
        "wqs", [128, NDT * HQ * HD + 2 * WC], F16, kind="ExternalInput"
    )
    wks = nc.dram_tensor("wks", [128, NDT * HD], F16, kind="ExternalInput")
    wvs = nc.dram_tensor("wvs", [128, NDT * HD], F16, kind="ExternalInput")
    kv = nc.dram_tensor("kv", [B, 128, 2 * T + WC], F16, kind="ExternalInput")
    outT = nc.dram_tensor("outT", [128, NDT * B], F32, kind="ExternalOutput")

    with tile.TileContext(nc) as tc:
        _emit(nc, tc, xs, wqs, wks, wvs, kv, outT)
    nc.compile()
    return nc


def _emit(nc, tc, xs, wqs, wks, wvs, kv, outT):
    from contextlib import ExitStack

    with ExitStack() as ctx:
        const = ctx.enter_context(tc.tile_pool(name="const", bufs=1))

        # x^T in f16: [128, (dt b)], host pre-swizzled, contiguous load
        xs_h = const.tile([128, NDT * B], F16, tag="xs_h")
        nc.sync.dma_start(xs_h[:], xs[:])

        # wk/wv resident f16 [128, (dt n)]
        wk_h = const.tile([128, NDT * HD], F16, tag="wk_h")
        nc.sync.dma_start(wk_h[:], wks[:])
        wv_h = const.tile([128, NDT * HD], F16, tag="wv_h")
        nc.sync.dma_start(wv_h[:], wvs[:])

        QTh = const.tile([128, HQ * B], F16, tag="QTh")     # q^T [d,(h,b)] f16
        KTnh = const.tile([128, B], F16, tag="KTnh")        # new-token K^T f16
        vrowh = const.tile([1, B * HD], F16, tag="vrowh")   # new-token V rows f16
        AT = const.tile([128, HQ * B], F16, tag="AT")       # attn out^T f16
        wo_h = const.tile([128, HQ * DIM], F16, tag="wo_h") # resident f16 wo
        vn_h = const.tile([B, HD], F16, tag="vn_h")
        outs = const.tile([128, NDT * B], F32, tag="outs")  # outT staging
        ones_h = const.tile([128, 1], F16, tag="ones_h")    # denominator lhsT
        ones_r = const.tile([1, 128], F32, tag="ones_r")    # bcast-matmul lhsT

        nc.vector.memset(ones_h[:], 1.0)
        nc.vector.memset(ones_r[:], 1.0)

        # ---------------- phase 0: projections (f16 PE, weights stationary) --
        with tc.tile_pool(name="psum0", bufs=1, space="PSUM") as pp0:
            qtp = [
                pp0.tile([128, B], F32, tag=f"qtp{h}", name=f"qtp{h}")
                for h in range(HQ)
            ]
            ktp = pp0.tile([128, B], F32, tag="ktp")
            vnp = pp0.tile([B, HD], F32, tag="vnp")

            # one wq load (+2 embedded wo chunks): 36KB/partition contiguous.
            # Resident in const: a closable pool would let kvpool reuse the
            # address space, which serializes KV0's write behind the
            # q-projection reads (~8us stall at the head of the stream).
            NWQ = NDT * HQ * HD
            wq_h = const.tile([128, NWQ + 2 * WC], F16, tag="wq_h")
            nc.sync.dma_start(wq_h[:], wqs[:])
            nc.vector.tensor_copy(
                wo_h[:, 14 * WC:16 * WC], wq_h[:, NWQ:NWQ + 2 * WC]
            )
            for h in range(HQ):
                for dt in range(NDT):
                    nc.tensor.matmul(
                        qtp[h][:],
                        wq_h[:, dt * HQ * HD + h * HD:
                             dt * HQ * HD + (h + 1) * HD],
                        xs_h[:, dt * B:(dt + 1) * B],
                        start=(dt == 0), stop=(dt == NDT - 1),
                    )
            for dt in range(NDT):
                nc.tensor.matmul(
                    ktp[:], wk_h[:, dt * HD:(dt + 1) * HD],
                    xs_h[:, dt * B:(dt + 1) * B],
                    start=(dt == 0), stop=(dt == NDT - 1),
                )
            for dt in range(NDT):
                nc.tensor.matmul(
                    vnp[:], xs_h[:, dt * B:(dt + 1) * B],
                    wv_h[:, dt * HD:(dt + 1) * HD],
                    start=(dt == 0), stop=(dt == NDT - 1),
                )

            for h in range(HQ):
                nc.vector.tensor_copy(QTh[:, h * B:(h + 1) * B], qtp[h][:])
            nc.vector.tensor_copy(KTnh[:], ktp[:])
            nc.vector.tensor_copy(vn_h[:], vnp[:])

            # v_new rows flattened onto partition 0 (HWDGE SBUF->SBUF; no
            # SWDGE anywhere -> SDMA engine 15 keeps its full bandwidth).
            # On the ACT ring: its data-dependency wait (v-projection chain)
            # must not stall the SP ring ahead of the first KV load.
            nc.scalar.dma_start(
                vrowh[:].rearrange("p (b c) -> p b c", c=HD)[0:1, :, :],
                vn_h[:],
            )

        # ---------------- phase 1: attention over the cache ----------------
        QTh3 = QTh[:].rearrange("p (h b) -> p b h", b=B)   # [128, b, 4]
        vrowh3 = vrowh[:].rearrange("p (b c) -> p b c", c=HD)
        AT3 = AT[:].rearrange("p (h b) -> p b h", b=B)

        with (
            tc.tile_pool(name="kvpool", bufs=3) as kvpool,
            tc.tile_pool(name="ptpool", bufs=2) as ptpool,
            tc.tile_pool(name="small", bufs=2) as small,
            tc.tile_pool(name="stpsum", bufs=2, space="PSUM") as stpsum,
            tc.tile_pool(name="opsum", bufs=2, space="PSUM") as opsum,
            tc.tile_pool(name="denpsum", bufs=1, space="PSUM") as denpsum,
            tc.tile_pool(name="rcbpsum", bufs=1, space="PSUM") as rcbpsum,
        ):
            # Batches 0..13: one fused 4MiB KV load each on the SP ring
            # (32KB/partition -> 32KB packets, best per-engine rate).
            # Batches 14..15: K and V split, issued on the second HWDGE
            # ring (ACT).  Their triggers sit behind exp(13)/exp(14) in ACT
            # program order, which serializes the tail loads in exactly
            # need-order right behind the SP stream's end.
            for b in range(B):
                kvb = kvpool.tile([128, 2 * T + WC], F16, tag="kvb",
                                  name=f"kvb{b}")
                if b < 14:
                    nc.sync.dma_start(kvb[:], kv[b])
                    # unpack this load's embedded wo chunk (rides the fused
                    # DMA so the scheduler cannot reorder it to the end)
                    nc.vector.tensor_copy(
                        wo_h[:, b * WC:(b + 1) * WC],
                        kvb[:, 2 * T:2 * T + WC],
                    )
                elif b == 14:
                    nc.scalar.dma_start(kvb[:, 0:T], kv[b][:, 0:T])
                    nc.scalar.dma_start(kvb[:, T:2 * T], kv[b][:, T:2 * T])
                else:
                    # last batch: V in two halves so the PV matmuls for
                    # tiles 0..31 overlap the second half's stream — the
                    # only load whose completion latency is fully exposed
                    nc.scalar.dma_start(kvb[:, 0:T], kv[b][:, 0:T])
                    nc.scalar.dma_start(kvb[:, T:T + T // 2],
                                        kv[b][:, T:T + T // 2])
                    nc.scalar.dma_start(kvb[:, T + T // 2:2 * T],
                                        kv[b][:, T + T // 2:2 * T])
                ktb = kvb[:, 0:T]
                vb = kvb[:, T:2 * T]

                # scores^T tiles: [t'(128), h(4)] per cache tile + new token
                stp = stpsum.tile([128, SW], F32, tag="stp")
                qb = QTh3[:, b, :]
                nc.tensor.matmul(
                    stp[0:1, 4 * NT:SW], KTnh[:, b:b + 1], qb,
                    start=True, stop=True,
                )
                for n in range(NT):
                    nc.tensor.matmul(
                        stp[:, 4 * n:4 * n + 4],
                        ktb[:, 128 * n:128 * (n + 1)],
                        qb,
                        start=True, stop=True,
                    )

                pt = ptpool.tile([128, SW], F16, tag="pt")
                nc.scalar.activation(pt[:, 0:4 * NT], stp[:, 0:4 * NT], Exp,
                                     scale=SCALE)
                nc.scalar.activation(
                    pt[0:1, 4 * NT:SW], stp[0:1, 4 * NT:SW], Exp, scale=SCALE,
                )

                # softmax denominators first (need only pt, not V): by the
                # time V lands the reciprocal broadcast is already done.
                dps = denpsum.tile([1, SW], F32, tag="dps")
                nc.tensor.matmul(
                    dps[0:1, 0:4 * NT], ones_h[:], pt[:, 0:4 * NT],
                    start=True, stop=True,
                )
                nc.tensor.matmul(
                    dps[0:1, 4 * NT:SW], ones_h[0:1, 0:1], pt[0:1, 4 * NT:SW],
                    start=True, stop=True,
                )
                dred = small.tile([1, HQ], F32, tag="dred")
                nc.vector.reduce_sum(
                    dred[:].rearrange("p h -> p h ()"),
                    dps[:].rearrange("p (g h) -> p h g", h=HQ),
                    axis=mybir.AxisListType.X,
                )
                rcr = small.tile([1, HQ], F32, tag="rcr")
                nc.vector.reciprocal(rcr[:], dred[:])
                # broadcast 1/den across partitions: ones[128] outer rcr[4]
                rcb = rcbpsum.tile([128, HQ], F32, tag="rcb")
                nc.tensor.matmul(rcb[:], ones_r[:], rcr[:],
                                 start=True, stop=True)
                rcs = small.tile([128, HQ], F32, tag="rcs")
                nc.vector.tensor_copy(rcs[:], rcb[:])

                # out [d(128), h(4)]: V tiles stationary (FWL), P moving
                op = opsum.tile([128, HQ], F32, tag="op")
                nc.tensor.matmul(
                    op[:], vrowh3[0:1, b, :], pt[0:1, 4 * NT:SW],
                    start=True, stop=False,
                )
                for n in range(NT):
                    nc.tensor.matmul(
                        op[:],
                        vb[:, 128 * n:128 * (n + 1)],
                        pt[:, 4 * n:4 * n + 4],
                        start=False, stop=(n == NT - 1),
                    )
                nc.vector.tensor_mul(AT3[:, b, :], op[:], rcs[:])

        # ---------------- phase 2: output projection (wo stationary) -------
        with tc.tile_pool(name="wopsum", bufs=4, space="PSUM") as wps:
            for nt in range(NDT):
                wop = wps.tile([128, B], F32, tag="wop")
                for cc in range(HQ):
                    nc.tensor.matmul(
                        wop[:],
                        wo_h[:, cc * DIM + nt * 128:cc * DIM + (nt + 1) * 128],
                        AT[:, cc * B:(cc + 1) * B],
                        start=(cc == 0), stop=(cc == HQ - 1),
                    )
                nc.vector.tensor_copy(outs[:, nt * B:(nt + 1) * B], wop[:])
                # stream the store out in 4 chunks as tiles complete
                if nt % 8 == 7:
                    g = nt // 8
                    nc.sync.dma_start(
                        outT[:, g * 8 * B:(g + 1) * 8 * B],
                        outs[:, g * 8 * B:(g + 1) * 8 * B],
                    )


_NC = None


def _get_nc():
    global _NC
    if _NC is None:
        _NC = _build_nc()
    return _NC


def _swz(w):
    """[DIM, n] -> [128, (dt n)] f16, dt-tile-major per partition."""
    n = w.shape[1]
    return np.ascontiguousarray(
        w.reshape(NDT, 128, n).transpose(1, 0, 2).reshape(128, NDT * n)
    ).astype(np.float16)


def make_in_maps(inputs):
    x = np.asarray(inputs["x"], dtype=np.float32).reshape(B, DIM)
    ck = np.asarray(inputs["cache_k"], dtype=np.float32)
    cv = np.asarray(inputs["cache_v"], dtype=np.float32)
    wq = np.asarray(inputs["wq"], dtype=np.float32)
    wk = np.asarray(inputs["wk"], dtype=np.float32)
    wv = np.asarray(inputs["wv"], dtype=np.float32)
    wo = np.asarray(inputs["wo"], dtype=np.float32)

    xs = _swz(np.ascontiguousarray(x.T))
    ck16 = ck.astype(np.float16)
    cv16 = cv.astype(np.float16)

    in_maps = []
    for c in range(NCORES):
        hq0 = HQ * HD * c
        wks = _swz(np.ascontiguousarray(wk[HD * c:HD * (c + 1), :].T))
        wvs = _swz(np.ascontiguousarray(wv[HD * c:HD * (c + 1), :].T))
        wos = np.ascontiguousarray(
            wo[:, hq0:hq0 + HQ * HD].T.reshape(HQ, 128, DIM)
            .transpose(1, 0, 2).reshape(128, HQ * DIM)
        ).astype(np.float16)
        # wq with wo chunks 14..15 appended; KV with wo chunk b appended
        wqs = np.ascontiguousarray(np.concatenate(
            [_swz(np.ascontiguousarray(wq[hq0:hq0 + HQ * HD, :].T)),
             wos[:, 14 * WC:16 * WC]], axis=1
        ))
        # K^T with columns permuted to the t = 64*p + n interleaved order
        # (matches V's natural contiguous-load partition mapping).
        a = ck16[:, :, c, :].reshape(B, 128, NT, HD)       # [b, p, n, d]
        kT_c = a.transpose(0, 3, 2, 1).reshape(B, HD, T)   # [b, d, 128n+p]
        v_c = np.ascontiguousarray(cv16[:, :, c, :]).reshape(B, 128, T)
        wo3 = wos[:, 0:B * WC].reshape(128, B, WC).transpose(1, 0, 2)
        kv_c = np.ascontiguousarray(
            np.concatenate([kT_c, v_c, wo3], axis=2)
        )
        in_maps.append({
            "xs": xs, "wqs": wqs, "wks": wks, "wvs": wvs,
            "kv": kv_c,
        })
    return in_maps


def gather(results):
    """Sum per-core outT partials and undo the [n-tile, b] transpose."""
    acc = np.zeros((B, DIM), dtype=np.float64)
    for r in results:
        o = np.asarray(r["outT"], dtype=np.float64)       # [128, (nt b)]
        acc += o.reshape(128, NDT, B).transpose(2, 1, 0).reshape(B, DIM)
    return acc


def run(in_maps, trace=False):
    nc = _get_nc()
    return run_bass_kernel_spmd(nc, in_maps, list(range(NCORES)), trace=trace)


def kernel(**inputs):
    res = run(make_in_maps(inputs)).results
    return gather(res).astype(np.float32).reshape(B, 1, DIM)
